# revision 46
# baseline (speedup 1.0000x reference)
"""Block-sparse attention kernel for Trainium2 (8 NeuronCores, SPMD).

Strategy
--------
* Shard batch*heads (2*16 = 32 pairs) across 8 cores, 4 heads per core.
* Per head, flash-style attention computed in S^T layout: scores are
  produced as S^T[k, q] (k on partitions, q on the free dim) via
  matmul(lhsT=K^T chunk, rhs=Q^T).  exp(sm_scale * S^T) runs on the
  scalar engine straight out of PSUM.  The PV matmul uses V (with an
  appended ones-column) as the stationary operand, accumulating
  O^T[d, q] plus the softmax denominators in one accumulation group.
  No max-subtraction is needed: scores are bounded (|s| << 88) and
  masked entries are exactly 0 (the reference uses finfo.min masking,
  which underflows to 0 after softmax's exp as well).
* Sparsity: the host reads row_starts/row_ends and compiles a schedule
  that (a) skips k-chunks that no query in the q-block attends to,
  (b) trims the q-range of the score matmul + exp per chunk, and
  (c) applies precomputed boundary masks (multiplicative 0/1 strips)
  only where row boundaries fall strictly inside a 128-wide k-chunk.
* Fully-masked rows (row_end <= row_start) are patched on the host with
  the uniform-softmax result (mean over all values), matching the
  reference's softmax-over-all-minimums behaviour.

The q/k inputs are pre-transposed on the host (d-major, replicated into
both partition halves) so the device performs no input transposes and
chunk pairs run as concurrent row-tiled K=64 matmuls.  All matmuls use
float32r (single-pass fp32, ~11 mantissa bits, 4x the throughput of
exact fp32).  The output transpose O^T -> O runs on the tensor engine
per 128-query tile in one batched end-phase, normalized by the
reciprocal denominators on the vector + scalar engines.
"""

import numpy as np

import concourse.mybir as mybir
import concourse.tile as tile
from concourse import bacc
from concourse.bass_utils import run_bass_kernel_spmd

F32 = mybir.dt.float32
F32R = mybir.dt.float32r           # single-pass reduced-precision fp32 (~11 mantissa bits)

B, H, N, D = 2, 16, 2048, 64
NCORES = 8
HPC = (B * H) // NCORES        # heads per core
CHUNK = 128                    # k-chunk (partition dim of S^T)
QP = 1024                      # q extent per pass
NPASS = N // QP
NCHUNK = N // CHUNK
MMF = 512                      # max fp32 matmul free dim



def _runs(mask):
    """Maximal [a, b) runs of True in a 1-D bool array."""
    idx = np.flatnonzero(np.diff(np.concatenate(([False], mask, [False])).astype(np.int8)))
    return list(zip(idx[0::2], idx[1::2]))


def _schedule(starts, ends):
    """Per (pass, chunk) work description, shared by all heads/cores."""
    sched = []
    for p in range(NPASS):
        qb = p * QP
        ps = starts[qb:qb + QP]
        pe = ends[qb:qb + QP]
        chunks = []
        for c in range(NCHUNK):
            lo, hi = c * CHUNK, (c + 1) * CHUNK
            allowed = (pe > lo) & (ps < hi)
            if not allowed.any():
                continue
            dis = _runs(~allowed)
            # trim leading/trailing fully-disallowed cols out of S/exp.
            # fp32r matmuls need even free offsets/counts, so snap outward
            # and zero the extra disallowed column(s) explicitly.
            qa = dis[0][1] if dis and dis[0][0] == 0 else 0
            qz = dis[-1][0] if dis and dis[-1][1] == QP else QP
            qa_e, qz_e = int(qa) & ~1, min(QP, (int(qz) + 1) & ~1)
            me = _runs(allowed & (pe > lo) & (pe < hi))
            ms = _runs(allowed & (ps > lo) & (ps < hi))
            # interior disallowed spans (inside [qa, qz)) are read by the
            # trimmed PV matmul and must be zeroed; the leading/trailing
            # spans only matter for the first chunk, whose PV is full-width
            interior = [(int(a), int(b)) for a, b in dis if a != 0 and b != QP]
            for a, b in ((qa_e, qa), (qz, qz_e)):
                if a < b:
                    interior.append((int(a), int(b)))
            qa, qz = qa_e, qz_e
            chunks.append(dict(c=c, qa=int(qa), qz=int(qz),
                               memsets=[(int(a), int(b)) for a, b in dis],
                               interior=interior,
                               mule=[(int(a), int(b)) for a, b in me],
                               muls=[(int(a), int(b)) for a, b in ms]))
        sched.append(chunks)
    return sched


def _build_program(sched, sm_scale, use_me, use_ms):
    nc = bacc.Bacc("TRN2", target_bir_lowering=False, debug=True)

    U32 = mybir.dt.uint32
    # kt/qt are replicated into both partition halves so pairs of k-chunks
    # run as two concurrent row-tiled K=64 matmuls (row groups 0-1 / 2-3)
    kt_h = nc.declare_dram_parameter("kt", [HPC, 128, N], F32R, isOutput=False)
    qt_h = nc.declare_dram_parameter("qt", [HPC, 128, N], F32R, isOutput=False)
    ve_h = nc.declare_dram_parameter("ve", [HPC, 128, NCHUNK * (D + 1)], F32R, isOutput=False)
    me_h = nc.declare_dram_parameter("me", [128, N], F32R, isOutput=False)
    ms_h = nc.declare_dram_parameter("ms", [128, N], F32R, isOutput=False)
    id_h = nc.declare_dram_parameter("ident", [128, 128], F32R, isOutput=False)
    o_h = nc.declare_dram_parameter("o", [HPC, N, D], F32, isOutput=True)

    exp_f = mybir.ActivationFunctionType.Exp

    with tile.TileContext(nc) as tc:
        with (
            tc.tile_pool(name="singles", bufs=1) as singles,
            tc.tile_pool(name="heads", bufs=3) as heads,
            tc.tile_pool(name="pbuf", bufs=8) as pbuf,
            tc.tile_pool(name="fin", bufs=6) as fin,
            tc.tile_pool(name="fstash", bufs=NPASS * HPC) as fstash,
            tc.tile_pool(name="spsum", bufs=3, space="PSUM") as spsum,
            tc.tile_pool(name="opsum", bufs=1, space="PSUM") as opsum,
        ):
            # flatten every (head, pass, chunk) into one continuous stream so
            # the pair pipeline never breaks at pass or head boundaries
            items = []
            head_sb = {}
            for g in range(HPC):
                for p in range(NPASS):
                    chunks = sched[p]
                    for idx, ch in enumerate(chunks):
                        items.append(dict(g=g, p=p, ch=ch, first=idx == 0,
                                          last=idx == len(chunks) - 1))

            def load_head(g):
                # kt via the SP ring and qt via the ACT ring so the two big head
                # DMAs run in parallel HWDGE FIFOs.  Head 0's loads are split so
                # the first chunk pair's slices land (and unblock the PE) first.
                kt_sb = heads.tile([128, N], F32R, tag="kt", name=f"kt_{g}")
                qt_sb = heads.tile([128, N], F32R, tag="qt", name=f"qt_{g}")
                if g == 0:
                    nc.sync.dma_start(out=kt_sb[:, 0:2 * CHUNK],
                                      in_=kt_h[g, :, 0:2 * CHUNK])
                    nc.scalar.dma_start(out=qt_sb[:, 0:QP], in_=qt_h[g, :, 0:QP])
                    nc.sync.dma_start(out=kt_sb[:, 2 * CHUNK:],
                                      in_=kt_h[g, :, 2 * CHUNK:])
                    nc.scalar.dma_start(out=qt_sb[:, QP:], in_=qt_h[g, :, QP:])
                else:
                    nc.sync.dma_start(out=kt_sb, in_=kt_h[g, :, :])
                    nc.scalar.dma_start(out=qt_sb, in_=qt_h[g, :, :])
                ve_sb = heads.tile([128, NCHUNK * (D + 1)], F32R, tag="ve",
                                   name=f"ve_{g}")
                nc.gpsimd.dma_start(out=ve_sb, in_=ve_h[g, :, :])
                head_sb[g] = (kt_sb, qt_sb, ve_sb)

            stash = []
            o_tiles = {}

            def emit_pv(it, p_sb):
                g, p, ch = it["g"], it["p"], it["ch"]
                if (g, p) not in o_tiles:
                    o_tiles[(g, p)] = opsum.tile([D + 1, QP], F32, tag="o",
                                                 name=f"o_{g}_{p}")
                o_ps = o_tiles[(g, p)]
                ve_sb = head_sb[g][2]
                c = ch["c"]
                for a in range(0, QP, MMF):
                    if it["first"]:
                        lo, hi = a, a + MMF
                    else:
                        lo, hi = max(a, ch["qa"]), min(a + MMF, ch["qz"])
                    if lo < hi:
                        nc.tensor.matmul(
                            o_ps[:, lo:hi],
                            lhsT=ve_sb[:, c * (D + 1):(c + 1) * (D + 1)],
                            rhs=p_sb[:, lo:hi],
                            start=it["first"], stop=it["last"],
                        )
                if it["last"]:
                    # free the o accumulator; transpose/normalize is stashed
                    # for the pipelined end-phase
                    f_sb = fstash.tile([D + 1, QP], F32R, tag="f",
                                       name=f"f_{g}_{p}")
                    nc.vector.tensor_copy(f_sb, o_ps)
                    stash.append((g, p, f_sb))
                    del o_tiles[(g, p)]

            # PE warmup on memset data: starts the tensor engine's I-fetch and
            # HAM clock ramp while the first DMAs are still in flight
            w_sb = singles.tile([128, MMF], F32R, tag="warm")
            nc.gpsimd.memset(w_sb.bitcast(U32), 0)
            for wi in range(14):
                w_ps = spsum.tile([128, QP], F32, tag="s", name=f"w_{wi}")
                nc.tensor.matmul(w_ps[:, 0:MMF], lhsT=w_sb[:, 0:128],
                                 rhs=w_sb, start=True, stop=True)

            # head 0's tensors gate the first matmuls — their DMAs go first
            load_head(0)
            me_sb = ms_sb = None
            if use_me:
                me_sb = singles.tile([128, N], F32R, tag="me")
                nc.sync.dma_start(out=me_sb, in_=me_h[:, :])
            if use_ms:
                ms_sb = singles.tile([128, N], F32R, tag="ms")
                nc.sync.dma_start(out=ms_sb, in_=ms_h[:, :])
            id_sb = singles.tile([128, 128], F32R, tag="ident")
            nc.sync.dma_start(out=id_sb, in_=id_h[:, :])
            pending = []
            for j0 in range(0, len(items), 2):
                pair = items[j0:j0 + 2]
                # stagger head loads: kick off head g+1's DMAs as soon as
                # head g's first pair is in flight
                g_hi = max(it["g"] for it in pair)
                if g_hi + 1 < HPC and g_hi + 1 not in head_sb:
                    load_head(g_hi + 1)
                sub = []
                tiles = []
                for k, it in enumerate(pair):
                    ch = it["ch"]
                    g, p = it["g"], it["p"]
                    s_ps = spsum.tile([128, QP], F32, tag="s",
                                      name=f"s_{j0}_{k}")
                    tiles.append(s_ps)
                    pp = 64 * k
                    qb = p * QP
                    mms = []
                    for a in range(0, QP, MMF):
                        lo, hi = max(a, ch["qa"]), min(a + MMF, ch["qz"])
                        if lo < hi:
                            mms.append((s_ps, pp, it, lo, hi))
                    sub.append(mms)
                # interleave A/B sub-matmuls for row-group concurrency
                for pr in [x for tup in __import__("itertools")
                           .zip_longest(*sub) for x in tup if x]:
                    s_ps, pp, it, lo, hi = pr
                    g, p, c = it["g"], it["p"], it["ch"]["c"]
                    kt_sb, qt_sb, _ = head_sb[g]
                    qb = p * QP
                    nc.tensor.matmul(
                        s_ps[:, lo:hi],
                        lhsT=kt_sb[pp:pp + 64, c * CHUNK:(c + 1) * CHUNK],
                        rhs=qt_sb[pp:pp + 64, qb + lo:qb + hi],
                        start=True, stop=True,
                        tile_position=(pp, 0),
                    )
                cur = []
                for k, it in enumerate(pair):
                    ch = it["ch"]
                    qb = it["p"] * QP
                    p_sb = pbuf.tile([128, QP], F32R, tag="p",
                                     name=f"p_{j0}_{k}")
                    nc.scalar.activation(p_sb[:, ch["qa"]:ch["qz"]],
                                         tiles[k][:, ch["qa"]:ch["qz"]],
                                         exp_f, scale=sm_scale)
                    for a, b in (ch["memsets"] if it["first"] else ch["interior"]):
                        nc.gpsimd.memset(p_sb[:, a:b].bitcast(U32), 0)
                    # boundary masks alternate between DVE and GpSimd so the
                    # exp -> mask -> PV chain isn't serialized on one engine
                    for mi, (a, b, m_sb) in enumerate(
                            [(a, b, me_sb) for a, b in ch["mule"]]
                            + [(a, b, ms_sb) for a, b in ch["muls"]]):
                        eng = nc.vector if (j0 + k + mi) % 2 else nc.gpsimd
                        eng.tensor_mul(p_sb[:, a:b], p_sb[:, a:b],
                                       m_sb[:, qb + a:qb + b])
                    cur.append((it, p_sb))
                for it, p_sb in pending:
                    emit_pv(it, p_sb)
                pending = cur
            for it, p_sb in pending:
                emit_pv(it, p_sb)

            # end-phase: transpose O^T -> O, normalize by the denominators,
            # store.  Transposes land 4-up in one PSUM bank; one strided
            # reciprocal covers the 4 denominator columns; scales alternate
            # between DVE and the otherwise-idle scalar engine.
            copy_f = mybir.ActivationFunctionType.Copy
            GRP = 4
            for n, (g, p, f_sb) in enumerate(stash):
                qb = p * QP
                oo_sb = fin.tile([128, (QP // 128) * D], F32, tag="oo",
                                 name=f"oo_{g}_{p}")
                for t0 in range(0, QP // 128, GRP):
                    gi = t0 // GRP
                    pool, tg = (spsum, "s") if (n * 2 + gi) % 4 else (opsum, "o")
                    t_ps = pool.tile([128, GRP * (D + 2)], F32R, tag=tg,
                                     name=f"t_{g}_{p}_{t0}")
                    for t in range(GRP):
                        # D+2 output cols: fp32r transpose needs an even
                        # innermost count; the extra identity column is zero
                        nc.tensor.transpose(
                            t_ps[:, t * (D + 2):(t + 1) * (D + 2)],
                            f_sb[:, (t0 + t) * 128:(t0 + t + 1) * 128],
                            id_sb[:D + 1, :D + 2])
                    r_sb = fin.tile([128, GRP], F32, tag="r", name=f"r_{g}_{p}_{t0}")
                    nc.vector.reciprocal(
                        r_sb, t_ps.rearrange("q (t c) -> q t c", c=D + 2)[:, :, D])
                    for t in range(GRP):
                        args = (oo_sb[:, (t0 + t) * D:(t0 + t + 1) * D],
                                t_ps[:, t * (D + 2):t * (D + 2) + D])
                        if t % 2:
                            nc.vector.tensor_scalar_mul(*args, r_sb[:, t:t + 1])
                        else:
                            nc.scalar.activation(*args, copy_f,
                                                 scale=r_sb[:, t:t + 1])
                nc.sync.dma_start(
                    out=o_h[g, qb:qb + QP, :].rearrange("(t p) d -> p t d", p=128),
                    in_=oo_sb.rearrange("p (t d) -> p t d", d=D),
                )

    nc.compile()
    return nc


_CACHE = {}


def _get_program(starts, ends, sm_scale, use_me, use_ms):
    key = (starts.tobytes(), ends.tobytes(), float(sm_scale), use_me, use_ms)
    if key not in _CACHE:
        sched = _schedule(starts, ends)
        _CACHE[key] = _build_program(sched, float(sm_scale), use_me, use_ms)
    return _CACHE[key]


def _prep_inputs(q, k, v, starts, ends, use_me, use_ms):
    """Per-core input dicts."""
    qf = np.asarray(q, np.float32).reshape(B * H, N, D)
    kf = np.asarray(k, np.float32).reshape(B * H, N, D)
    vf = np.asarray(v, np.float32).reshape(B * H, N, D)

    # boundary mask strips (shared across heads): column j holds the
    # within-chunk prefix/suffix mask for row_ends[j]/row_starts[j]
    rows = np.arange(128, dtype=np.int64)[:, None]
    me = (rows < (ends[None, :] % CHUNK)).astype(np.float32)
    ms = (rows >= (starts[None, :] % CHUNK)).astype(np.float32)
    ident = np.eye(128, dtype=np.float32)

    in_maps = []
    for i in range(NCORES):
        sl = slice(i * HPC, (i + 1) * HPC)
        kt1 = kf[sl].transpose(0, 2, 1)                      # [HPC, D, N]
        qt1 = qf[sl].transpose(0, 2, 1)
        kt = np.ascontiguousarray(np.concatenate([kt1, kt1], axis=1))
        qt = np.ascontiguousarray(np.concatenate([qt1, qt1], axis=1))
        ve = np.ones([HPC, 128, NCHUNK, D + 1], np.float32)
        ve[:, :, :, :D] = vf[sl].reshape(HPC, NCHUNK, CHUNK, D).transpose(0, 2, 1, 3)
        ve = np.ascontiguousarray(ve.reshape(HPC, 128, NCHUNK * (D + 1)))
        in_maps.append({"kt": kt, "qt": qt, "ve": ve, "me": me, "ms": ms,
                        "ident": ident})
    return in_maps


def _run(inputs, trace=False):
    q, k, v = inputs["q"], inputs["k"], inputs["v"]
    sm_scale = float(np.asarray(inputs["sm_scale"]))
    starts_raw = np.asarray(inputs["row_starts"], np.int64)
    ends_raw = np.asarray(inputs["row_ends"], np.int64)
    starts = np.clip(starts_raw, 0, N)
    ends = np.clip(ends_raw, 0, N)

    use_ms = bool((starts % CHUNK).any())
    use_me = bool(((ends % CHUNK) * (ends > starts)).any())

    nc = _get_program(starts, ends, sm_scale, use_me, use_ms)
    in_maps = _prep_inputs(q, k, v, starts, ends, use_me, use_ms)
    res = run_bass_kernel_spmd(nc, in_maps, list(range(NCORES)), trace=trace)

    out = np.empty([B * H, N, D], np.float32)
    for i in range(NCORES):
        out[i * HPC:(i + 1) * HPC] = res.results[i]["o"]
    out = out.reshape(B, H, N, D)

    empty = ends <= starts
    if empty.any():
        mean_v = np.asarray(v, np.float32).mean(axis=2)          # [B, H, D]
        out[:, :, empty, :] = mean_v[:, :, None, :]
    return out, res.exec_time_ns


def kernel(**inputs) -> np.ndarray:
    out, _ = _run(inputs, trace=False)
    return out
